# revision 2
# baseline (speedup 1.0000x reference)
"""Trainium2 Bass kernel for ChannelAwareAttentionModule.

Reference computation (per sample b, x: (256, 4096) = (C, H*W)):
    g     = relu(BN(Wg  @ x))                  (128, 4096)
    theta = relu(BN(Wth @ x))                  (128, 4096)
    phi   = relu(BN(Wph @ x))                  (128, 4096)
    f     = softmax(theta @ phi.T, axis=-1)    (128, 128)
    y     = f @ g                              (128, 4096)
    z     = y.T.reshape(128, 4096)             (torch permute+view scramble)
    out   = BN(Ww @ z) + x                     (256, 4096)

Sharding: pure data-parallel, 2 samples per core on 8 cores.

Kernel-level transformations:
  * BN folded into conv weights/biases on the host; operands cast to fp16
    (fp32 accumulation in PSUM throughout).
  * Final-proj bias folded into x (x_adj = x + bw), projection biases
    compensated; residual (+ x_adj) added during final evacuation (DVE
    scalar_tensor_tensor) or via identity-weight matmul (ACT copy path).
  * theta/phi computed directly in n-transposed layout (needed by the
    scores matmul); their (free-dim-varying) biases enter via rank-1
    K=1 matmuls that also open the PSUM banks (start=True).
  * The permute+view scramble z[c', q*128+r] = y[r, 32c'+q] is realized by
    a stride-32 lhsT access on g:
    z[:, q*128:(q+1)*128] = matmul(lhsT=g[:, q::32], rhs=f.T)
    so no transpose of y is ever materialized.

Scheduling (v2): the PE instruction stream is kept dense to hold the HAM
clock gate at 2.4 GHz: the two samples are software-pipelined so sample
b0's softmax/attention runs interleaved inside sample b1's projections,
and all PSUM tiles rotate through one 3-buffer (128,1024) pool so no
matmul waits on an evacuation.
"""

from contextlib import ExitStack

import numpy as np

import concourse.bacc as bacc
import concourse.mybir as mybir
from concourse import tile
from concourse.bass_utils import run_bass_kernel_spmd

F32 = mybir.dt.float32
F16 = mybir.dt.float16
AF = mybir.ActivationFunctionType
ALU = mybir.AluOpType
AX = mybir.AxisListType

NCORES = 8
B, C, CI, N = 16, 256, 128, 4096
BPC = B // NCORES  # samples per core
NQ = N // 128  # 32 column blocks of 128
EPS = 1e-5


def _build_nc():
    nc = bacc.Bacc("TRN2", target_bir_lowering=False, debug=False, num_devices=NCORES)

    x_d = nc.dram_tensor("x", [BPC, C, N], F16, kind="ExternalInput")
    wall_d = nc.dram_tensor("wall", [128, 1792], F16, kind="ExternalInput")
    bg_d = nc.dram_tensor("bg", [CI, 1], F32, kind="ExternalInput")
    out_d = nc.dram_tensor("out", [BPC, C, N], F16, kind="ExternalOutput")

    with tile.TileContext(nc) as tc, ExitStack() as ctx:
        wpool = ctx.enter_context(tc.tile_pool(name="wts", bufs=1))
        xpool = ctx.enter_context(tc.tile_pool(name="xp", bufs=4 * BPC))
        tppool = ctx.enter_context(tc.tile_pool(name="tp", bufs=BPC))
        gpool = ctx.enter_context(tc.tile_pool(name="gp", bufs=BPC))
        zpool = ctx.enter_context(tc.tile_pool(name="zp", bufs=4))
        smpool = ctx.enter_context(tc.tile_pool(name="sm", bufs=2))
        opool = ctx.enter_context(tc.tile_pool(name="ost", bufs=4))
        # one shared rotating PSUM pool for thph/g/z/final (2-bank tiles)
        # plus a small pool for scores + f-transpose
        ps = ctx.enter_context(tc.tile_pool(name="ps", bufs=3, space="PSUM"))
        ps_sml = ctx.enter_context(tc.tile_pool(name="ps_sml", bufs=2, space="PSUM"))

        # --- replicated constants (single packed DMA + small bg) ---
        w_all = wpool.tile([128, 1792], F16, tag="w_all")
        b_g = wpool.tile([CI, 1], F32, tag="b_g")
        nc.sync.dma_start(w_all[:, :], wall_d[:])
        nc.sync.dma_start(b_g[:, 0:1], bg_d[:])
        w_g = w_all[:, 0:256]
        w_tp = w_all[:, 256:768]
        w_w = w_all[:, 768:1024]
        idf = w_all[:, 1024:1152]
        b_tp = w_all[0:1, 1152:1664]
        ones1 = w_all[0:1, 1664:1792]

        # evac engine round-robin (ACT / DVE)
        state = {"i": 0}

        def pick():
            state["i"] += 1
            return state["i"] % 2 == 0

        xs = {}
        thph = {}
        g_sb = {}
        g_v = {}
        f_sb = {}
        fT = {}

        # x tiles: (128, 2, 1024) per j-chunk; [:, k, :] = channels 128k..128k+127
        for b in range(BPC):
            xs[b] = [
                xpool.tile([128, 2, 1024], F16, tag="x", name=f"x_{b}_{j}")
                for j in range(4)
            ]
            thph[b] = tppool.tile([128, NQ * 256], F16, tag="thph", name=f"thph_{b}")
            g_sb[b] = gpool.tile([128, N], F16, tag="g_sb", name=f"g_sb_{b}")
            g_v[b] = g_sb[b][:, :].rearrange("p (c q) -> p q c", q=NQ)

        def load_x(b, j, split=False):
            src = x_d[b, :, 1024 * j : 1024 * (j + 1)].rearrange(
                "(k p) n -> p k n", k=2
            )
            if split:
                for k in range(2):
                    nc.sync.dma_start(xs[b][j][:, k, :], src[:, k, :])
            else:
                nc.sync.dma_start(xs[b][j][:, :, :], src)

        def proj_tile(b, t):
            """theta^T/phi^T projection for q-blocks 4t..4t+3 -> thph[b]."""
            pt = ps.tile([128, 1024], F32, tag="ps", name=f"pt_{b}_{t}")
            for half in range(2):
                nc.tensor.matmul(
                    pt[:, 512 * half : 512 * (half + 1)],
                    ones1,
                    b_tp,
                    start=True,
                    stop=False,
                    skip_group_check=True,
                )
            j, base = divmod(512 * t, 1024)
            for h in range(4):
                qc = base + 128 * h
                for k in range(2):
                    nc.tensor.matmul(
                        pt[:, 256 * h : 256 * (h + 1)],
                        xs[b][j][:, k, qc : qc + 128],
                        w_tp[:, 256 * k : 256 * (k + 1)],
                        start=False,
                        stop=(h == 3 and k == 1),
                        skip_group_check=True,
                    )
            dst = thph[b][:, 1024 * t : 1024 * (t + 1)]
            if pick():
                nc.scalar.activation(dst, pt[:, :], AF.Relu)
            else:
                nc.vector.tensor_scalar(dst, pt[:, :], 0.0, None, ALU.max)

        def g_tile(b, j):
            """g projection for n-cols 1024j..1024j+1023 -> g_sb[b]."""
            pg = ps.tile([128, 1024], F32, tag="ps", name=f"pg_{b}_{j}")
            for h2 in range(2):
                for k in range(2):
                    nc.tensor.matmul(
                        pg[:, 512 * h2 : 512 * (h2 + 1)],
                        w_g[:, 128 * k : 128 * (k + 1)],
                        xs[b][j][:, k, 512 * h2 : 512 * (h2 + 1)],
                        start=(k == 0),
                        stop=(k == 1),
                        skip_group_check=True,
                    )
            dst = g_sb[b][:, 1024 * j : 1024 * (j + 1)]
            if pick():
                nc.scalar.activation(dst, pg[:, :], AF.Relu, bias=b_g[:, 0:1])
            else:
                nc.vector.tensor_scalar(dst, pg[:, :], b_g[:, 0:1], 0.0, ALU.add, ALU.max)

        def scores(b):
            """scores + softmax -> f_sb[b] (PE part is just the 32 matmuls)."""
            ps_s = ps_sml.tile([128, 128], F32, tag="ps_sml", name=f"ps_s_{b}")
            for q in range(NQ):
                nc.tensor.matmul(
                    ps_s[:, :],
                    thph[b][:, 256 * q : 256 * q + 128],
                    thph[b][:, 256 * q + 128 : 256 * (q + 1)],
                    start=(q == 0),
                    stop=(q == NQ - 1),
                    skip_group_check=True,
                )
            negmax = smpool.tile([128, 1], F32, tag="negmax", name=f"negmax_{b}")
            nc.vector.reduce_max(negmax[:, :], ps_s[:, :], axis=AX.X, negate=True)
            e_sb = smpool.tile([128, 128], F16, tag="e_sb", name=f"e_sb_{b}")
            sumex = smpool.tile([128, 1], F32, tag="sumex", name=f"sumex_{b}")
            nc.scalar.activation(
                e_sb[:, :], ps_s[:, :], AF.Exp, bias=negmax[:, :], accum_out=sumex[:, :]
            )
            rs = smpool.tile([128, 1], F32, tag="rs", name=f"rs_{b}")
            nc.vector.reciprocal(rs[:, :], sumex[:, :])
            f_sb[b] = smpool.tile([128, 128], F16, tag="f_sb", name=f"f_sb_{b}")
            nc.scalar.activation(f_sb[b][:, :], e_sb[:, :], AF.Copy, scale=rs[:, :])

        def transpose_f(b):
            ps_t = ps_sml.tile([128, 128], F16, tag="ps_sml", name=f"ps_t_{b}")
            nc.tensor.transpose(ps_t[:, :], f_sb[b][:, :], idf[:, :])
            fT[b] = smpool.tile([128, 128], F16, tag="fT", name=f"fT_{b}")
            nc.vector.tensor_copy(fT[b][:, :], ps_t[:, :])

        def att_pair(b, tz):
            """z for q-blocks 8tz..8tz+7, then final proj + residual + DMA out
            for output columns 1024tz..1024tz+1023."""
            pz = ps.tile([128, 1024], F32, tag="ps", name=f"pz_{b}_{tz}")
            for qq in range(8):
                q = 8 * tz + qq
                nc.tensor.matmul(
                    pz[:, 128 * qq : 128 * (qq + 1)],
                    g_v[b][:, q, :],
                    fT[b][:, :],
                    start=True,
                    stop=True,
                    skip_group_check=True,
                )
            z2 = zpool.tile([128, 1024], F16, tag="z2", name=f"z2_{b}_{tz}")
            if pick():
                nc.scalar.copy(z2[:, :], pz[:, :])
            else:
                nc.vector.tensor_copy(z2[:, :], pz[:, :])

            otb = opool.tile([128, 2, 1024], F16, tag="ost", name=f"ot_{b}_{tz}")
            for u in range(2):
                pf = ps.tile([128, 1024], F32, tag="ps", name=f"pf_{b}_{tz}_{u}")
                pf_r = pf[:, :].rearrange("p (h n) -> p h n", h=2)
                dst = otb[:, :, 512 * u : 512 * (u + 1)]
                x_r = xs[b][tz][:, :, 512 * u : 512 * (u + 1)]
                if pick():
                    # residual via identity matmul, ACT copy out
                    for h in range(2):
                        nc.tensor.matmul(
                            pf[:, 512 * h : 512 * (h + 1)],
                            idf,
                            xs[b][tz][:, h, 512 * u : 512 * (u + 1)],
                            start=True,
                            stop=False,
                            skip_group_check=True,
                        )
                        nc.tensor.matmul(
                            pf[:, 512 * h : 512 * (h + 1)],
                            w_w[:, 128 * h : 128 * (h + 1)],
                            z2[:, 512 * u : 512 * (u + 1)],
                            start=False,
                            stop=True,
                            skip_group_check=True,
                        )
                    nc.scalar.copy(dst, pf_r)
                else:
                    # residual fused into the DVE evacuation
                    for h in range(2):
                        nc.tensor.matmul(
                            pf[:, 512 * h : 512 * (h + 1)],
                            w_w[:, 128 * h : 128 * (h + 1)],
                            z2[:, 512 * u : 512 * (u + 1)],
                            start=True,
                            stop=True,
                            skip_group_check=True,
                        )
                    nc.vector.scalar_tensor_tensor(
                        dst, pf_r, 0.0, x_r, ALU.add, ALU.add
                    )
            nc.sync.dma_start(
                out_d[b, :, 1024 * tz : 1024 * (tz + 1)].rearrange(
                    "(h p) n -> p h n", h=2
                ),
                otb[:, :, :],
            )

        # ================= schedule =================
        # DMAs: constants, then x chunks in consumption order.
        load_x(0, 0, split=True)
        for j in range(1, 4):
            load_x(0, j)
        for j in range(4):
            load_x(1, j)

        # sample 0 projections (bias matmuls run first and only need w_all)
        for t in range(8):
            proj_tile(0, t)
            if t % 2 == 1:
                g_tile(0, t // 2)
        scores(0)

        # sample 1 projections with sample-0 attention interleaved
        for j in range(4):
            proj_tile(1, 2 * j)
            if j == 0:
                transpose_f(0)
            proj_tile(1, 2 * j + 1)
            g_tile(1, j)
            if j >= 1:
                att_pair(0, j - 1)
        scores(1)
        att_pair(0, 3)
        transpose_f(1)
        for tz in range(4):
            att_pair(1, tz)

    nc.compile()
    return nc


_CACHE = {}


def _prepare(inputs):
    """Fold BN into weights/biases and build per-core input maps."""

    def fold(w, bias, gamma, beta, mean, var):
        inv = gamma / np.sqrt(var + EPS)
        return (w * inv[:, None]).astype(np.float32), (
            beta + (bias - mean) * inv
        ).astype(np.float32)

    Wg, bg = fold(
        inputs["g_w"], inputs["g_b"], inputs["g_gamma"], inputs["g_beta"],
        inputs["g_mean"], inputs["g_var"],
    )
    Wth, bth = fold(
        inputs["th_w"], inputs["th_b"], inputs["th_gamma"], inputs["th_beta"],
        inputs["th_mean"], inputs["th_var"],
    )
    Wph, bph = fold(
        inputs["ph_w"], inputs["ph_b"], inputs["ph_gamma"], inputs["ph_beta"],
        inputs["ph_mean"], inputs["ph_var"],
    )
    Ww, bw = fold(
        inputs["w_w"], inputs["w_b"], inputs["w_gamma"], inputs["w_beta"],
        inputs["w_mean"], inputs["w_var"],
    )

    # x_adj = x + bw (per out-channel); compensate projection biases.
    x = np.asarray(inputs["x"], dtype=np.float32).reshape(B, C, N)
    x_adj = (x + bw[None, :, None]).astype(np.float16)
    bg_a = bg - Wg @ bw
    bth_a = bth - Wth @ bw
    bph_a = bph - Wph @ bw

    WgT = np.ascontiguousarray(Wg.T)  # (256, 128)
    wg_host = np.concatenate([WgT[0:128], WgT[128:256]], axis=1)  # (128, 256)
    WtpT = np.concatenate([Wth.T, Wph.T], axis=1)  # (256, 256)
    wtp_host = np.concatenate([WtpT[0:128], WtpT[128:256]], axis=1)  # (128, 512)
    btp_host = np.concatenate([bth_a, bph_a, bth_a, bph_a]).reshape(1, 512)
    ww_host = np.ascontiguousarray(Ww.T)  # (128, 256)

    wall = np.zeros((128, 1792), dtype=np.float16)
    wall[:, 0:256] = wg_host
    wall[:, 256:768] = wtp_host
    wall[:, 768:1024] = ww_host
    wall[:, 1024:1152] = np.eye(128, dtype=np.float16)
    wall[0, 1152:1664] = btp_host[0]
    wall[0, 1664:1792] = 1.0
    consts = {
        "wall": wall,
        "bg": np.ascontiguousarray(bg_a.reshape(CI, 1), dtype=np.float32),
    }
    in_maps = []
    for i in range(NCORES):
        m = dict(consts)
        m["x"] = np.ascontiguousarray(x_adj[BPC * i : BPC * (i + 1)])
        in_maps.append(m)
    return in_maps


def _get_nc():
    if "nc" not in _CACHE:
        _CACHE["nc"] = _build_nc()
    return _CACHE["nc"]


def run(inputs, **kw):
    """Run on hardware; returns (full_output, BassKernelResults)."""
    nc = _get_nc()
    in_maps = _prepare(inputs)
    res = run_bass_kernel_spmd(nc, in_maps, list(range(NCORES)), **kw)
    out = np.concatenate(
        [
            np.asarray(res.results[i]["out"], dtype=np.float32).reshape(BPC, C, 64, 64)
            for i in range(NCORES)
        ],
        axis=0,
    )
    return np.ascontiguousarray(out), res


def kernel(**inputs):
    out, _ = run(inputs)
    return out


# revision 4
# speedup vs baseline: 1.2692x; 1.2692x over previous
"""Trainium2 Bass kernel for ChannelAwareAttentionModule.

Reference computation (per sample b, x: (256, 4096) = (C, H*W)):
    g     = relu(BN(Wg  @ x))                  (128, 4096)
    theta = relu(BN(Wth @ x))                  (128, 4096)
    phi   = relu(BN(Wph @ x))                  (128, 4096)
    f     = softmax(theta @ phi.T, axis=-1)    (128, 128)
    y     = f @ g                              (128, 4096)
    z     = y.T.reshape(128, 4096)             (torch permute+view scramble)
    out   = BN(Ww @ z) + x                     (256, 4096)

Sharding: pure data-parallel, 2 samples per core on 8 cores.

Kernel-level transformations:
  * BN folded into conv weights/biases on the host.
  * Final-proj bias folded into x (x_adj = x + bw); residual added during
    the final evacuation (DVE scalar_tensor_tensor) or via identity-weight
    matmul (ACT copy path).
  * theta/phi computed directly in n-transposed layout; their free-dim
    biases enter via rank-1 K=1 matmuls that open the PSUM banks.
  * The permute+view scramble is realized by a stride-32 lhsT access on g.
  * dtypes: bf16 moving operands stream 2 elements/cycle through the PE
    array, so everything is bf16 EXCEPT the theta/phi weights (kept fp16:
    their quantization error enters the attention scores coherently over
    n and bf16 there pushes the final error to ~2.4e-2).
  * Scheduling: sample b0's attention is interleaved into sample b1's
    projections; z->final is software-pipelined (att_z / att_fin) so the
    final matmuls never wait on a fresh z evacuation; PSUM tiles rotate
    through one 3-buffer pool to keep the PE dense (HAM stays at 2.4 GHz).
"""

from contextlib import ExitStack

import ml_dtypes
import numpy as np

import concourse.bacc as bacc
import concourse.mybir as mybir
from concourse import tile
from concourse.bass_utils import run_bass_kernel_spmd

F32 = mybir.dt.float32
F16 = mybir.dt.float16
BF16 = mybir.dt.bfloat16
AF = mybir.ActivationFunctionType
ALU = mybir.AluOpType
AX = mybir.AxisListType

NCORES = 8
B, C, CI, N = 16, 256, 128, 4096
BPC = B // NCORES  # samples per core
NQ = N // 128  # 32 column blocks of 128
EPS = 1e-5


def _build_nc():
    nc = bacc.Bacc("TRN2", target_bir_lowering=False, debug=False, num_devices=NCORES)

    x_d = nc.dram_tensor("x", [BPC, C, N], BF16, kind="ExternalInput")
    wbf_d = nc.dram_tensor("wbf", [128, 1280], BF16, kind="ExternalInput")
    wtp_d = nc.dram_tensor("wtp16", [128, 640], F16, kind="ExternalInput")
    bg_d = nc.dram_tensor("bg", [CI, 1], F32, kind="ExternalInput")
    out_d = nc.dram_tensor("out", [BPC, C, N], F16, kind="ExternalOutput")

    with tile.TileContext(nc) as tc, ExitStack() as ctx:
        wpool = ctx.enter_context(tc.tile_pool(name="wts", bufs=1))
        xpool = ctx.enter_context(tc.tile_pool(name="xp", bufs=BPC))
        tppool = ctx.enter_context(tc.tile_pool(name="tp", bufs=BPC))
        gpool = ctx.enter_context(tc.tile_pool(name="gp", bufs=BPC))
        zpool = ctx.enter_context(tc.tile_pool(name="zp", bufs=4))
        smpool = ctx.enter_context(tc.tile_pool(name="sm", bufs=2))
        opool = ctx.enter_context(tc.tile_pool(name="ost", bufs=4))
        ps = ctx.enter_context(tc.tile_pool(name="ps", bufs=3, space="PSUM"))
        ps_sml = ctx.enter_context(tc.tile_pool(name="ps_sml", bufs=2, space="PSUM"))

        # --- replicated constants ---
        w_bf = wpool.tile([128, 1280], BF16, tag="w_bf")
        w_16 = wpool.tile([128, 640], F16, tag="w_16")
        b_g = wpool.tile([CI, 1], F32, tag="b_g")
        nc.sync.dma_start(w_bf[:, :], wbf_d[:])
        nc.sync.dma_start(w_16[:, :], wtp_d[:])
        nc.sync.dma_start(b_g[:, 0:1], bg_d[:])
        w_g = w_bf[:, 0:256]
        w_w = w_bf[:, 256:512]
        idf = w_bf[:, 512:640]  # bf16 identity (residual matmuls)
        b_tp = w_bf[0:1, 640:1152]
        ones1 = w_bf[0:1, 1152:1280]
        w_tp = w_16[:, 0:512]
        idf16 = w_16[:, 512:640]  # fp16 identity (f transpose)

        state = {"i": 0}

        def pick():
            state["i"] += 1
            return state["i"] % 2 == 0

        xs = {}
        thph = {}
        g_sb = {}
        g_v = {}
        f_sb = {}
        fT = {}
        z2 = {}

        for b in range(BPC):
            xs[b] = xpool.tile([128, 2, N], BF16, tag="x", name=f"x_{b}")
            thph[b] = tppool.tile([128, NQ * 256], F16, tag="thph", name=f"thph_{b}")
            g_sb[b] = gpool.tile([128, N], F16, tag="g_sb", name=f"g_sb_{b}")
            g_v[b] = g_sb[b][:, :].rearrange("p (c q) -> p q c", q=NQ)

        def load_x(b, j0, j1, split=False):
            src = x_d[b, :, 1024 * j0 : 1024 * j1].rearrange("(k p) n -> p k n", k=2)
            dst = xs[b][:, :, 1024 * j0 : 1024 * j1]
            if split:
                for k in range(2):
                    nc.sync.dma_start(dst[:, k, :], src[:, k, :])
            else:
                nc.sync.dma_start(dst, src)

        def proj_tile(b, t):
            """theta^T/phi^T projection for q-blocks 4t..4t+3 -> thph[b]."""
            pt = ps.tile([128, 1024], F32, tag="ps", name=f"pt_{b}_{t}")
            for half in range(2):
                nc.tensor.matmul(
                    pt[:, 512 * half : 512 * (half + 1)],
                    ones1,
                    b_tp,
                    start=True,
                    stop=False,
                    skip_group_check=True,
                )
            for h in range(4):
                qc = 512 * t + 128 * h
                for k in range(2):
                    nc.tensor.matmul(
                        pt[:, 256 * h : 256 * (h + 1)],
                        xs[b][:, k, qc : qc + 128],
                        w_tp[:, 256 * k : 256 * (k + 1)],
                        start=False,
                        stop=(h == 3 and k == 1),
                        skip_group_check=True,
                    )
            dst = thph[b][:, 1024 * t : 1024 * (t + 1)]
            if pick():
                nc.scalar.activation(dst, pt[:, :], AF.Relu)
            else:
                nc.vector.tensor_scalar(dst, pt[:, :], 0.0, None, ALU.max)

        def g_tile(b, j):
            """g projection for n-cols 1024j..1024j+1023 -> g_sb[b].
            k-outer order: consecutive matmuls share the stationary w_g chunk."""
            pg = ps.tile([128, 1024], F32, tag="ps", name=f"pg_{b}_{j}")
            for k in range(2):
                for h2 in range(2):
                    nc.tensor.matmul(
                        pg[:, 512 * h2 : 512 * (h2 + 1)],
                        w_g[:, 128 * k : 128 * (k + 1)],
                        xs[b][:, k, 1024 * j + 512 * h2 : 1024 * j + 512 * (h2 + 1)],
                        start=(k == 0),
                        stop=(k == 1),
                        skip_group_check=True,
                    )
            dst = g_sb[b][:, 1024 * j : 1024 * (j + 1)]
            if pick():
                nc.scalar.activation(dst, pg[:, :], AF.Relu, bias=b_g[:, 0:1])
            else:
                nc.vector.tensor_scalar(dst, pg[:, :], b_g[:, 0:1], 0.0, ALU.add, ALU.max)

        def scores(b):
            ps_s = ps_sml.tile([128, 128], F32, tag="ps_sml", name=f"ps_s_{b}")
            for q in range(NQ):
                nc.tensor.matmul(
                    ps_s[:, :],
                    thph[b][:, 256 * q : 256 * q + 128],
                    thph[b][:, 256 * q + 128 : 256 * (q + 1)],
                    start=(q == 0),
                    stop=(q == NQ - 1),
                    skip_group_check=True,
                )
            negmax = smpool.tile([128, 1], F32, tag="negmax", name=f"negmax_{b}")
            nc.vector.reduce_max(negmax[:, :], ps_s[:, :], axis=AX.X, negate=True)
            e_sb = smpool.tile([128, 128], F16, tag="e_sb", name=f"e_sb_{b}")
            sumex = smpool.tile([128, 1], F32, tag="sumex", name=f"sumex_{b}")
            nc.scalar.activation(
                e_sb[:, :], ps_s[:, :], AF.Exp, bias=negmax[:, :], accum_out=sumex[:, :]
            )
            rs = smpool.tile([128, 1], F32, tag="rs", name=f"rs_{b}")
            nc.vector.reciprocal(rs[:, :], sumex[:, :])
            f_sb[b] = smpool.tile([128, 128], F16, tag="f_sb", name=f"f_sb_{b}")
            nc.scalar.activation(f_sb[b][:, :], e_sb[:, :], AF.Copy, scale=rs[:, :])

        def transpose_f(b):
            ps_t = ps_sml.tile([128, 128], F16, tag="ps_sml", name=f"ps_t_{b}")
            nc.tensor.transpose(ps_t[:, :], f_sb[b][:, :], idf16[:, :])
            fT[b] = smpool.tile([128, 128], F16, tag="fT", name=f"fT_{b}")
            nc.vector.tensor_copy(fT[b][:, :], ps_t[:, :])

        def att_z(b, tz):
            """z for q-blocks 8tz..8tz+7 -> z2[b] (bf16)."""
            pz = ps.tile([128, 1024], F32, tag="ps", name=f"pz_{b}_{tz}")
            for qq in range(8):
                q = 8 * tz + qq
                nc.tensor.matmul(
                    pz[:, 128 * qq : 128 * (qq + 1)],
                    g_v[b][:, q, :],
                    fT[b][:, :],
                    start=(qq % 4 == 0),
                    stop=(qq % 4 == 3),
                    skip_group_check=True,
                )
            z2[(b, tz)] = zpool.tile([128, 1024], BF16, tag="z2", name=f"z2_{b}_{tz}")
            if pick():
                nc.scalar.copy(z2[(b, tz)][:, :], pz[:, :])
            else:
                nc.vector.tensor_copy(z2[(b, tz)][:, :], pz[:, :])

        def att_fin(b, tz):
            """final proj + residual + DMA out for out-cols 1024tz..1024tz+1023."""
            zz = z2[(b, tz)]
            otb = opool.tile([128, 2, 1024], F16, tag="ost", name=f"ot_{b}_{tz}")
            for u in range(2):
                pf = ps.tile([128, 1024], F32, tag="ps", name=f"pf_{b}_{tz}_{u}")
                pf_r = pf[:, :].rearrange("p (h n) -> p h n", h=2)
                dst = otb[:, :, 512 * u : 512 * (u + 1)]
                x_r = xs[b][:, :, 1024 * tz + 512 * u : 1024 * tz + 512 * (u + 1)]
                if u == 0:
                    # residual via identity matmul, ACT copy out
                    for h in range(2):
                        nc.tensor.matmul(
                            pf[:, 512 * h : 512 * (h + 1)],
                            idf,
                            x_r[:, h, :],
                            start=True,
                            stop=False,
                            skip_group_check=True,
                        )
                        nc.tensor.matmul(
                            pf[:, 512 * h : 512 * (h + 1)],
                            w_w[:, 128 * h : 128 * (h + 1)],
                            zz[:, 512 * u : 512 * (u + 1)],
                            start=False,
                            stop=True,
                            skip_group_check=True,
                        )
                    nc.scalar.copy(dst, pf_r)
                else:
                    # residual fused into the DVE evacuation
                    for h in range(2):
                        nc.tensor.matmul(
                            pf[:, 512 * h : 512 * (h + 1)],
                            w_w[:, 128 * h : 128 * (h + 1)],
                            zz[:, 512 * u : 512 * (u + 1)],
                            start=True,
                            stop=True,
                            skip_group_check=True,
                        )
                    nc.vector.scalar_tensor_tensor(dst, pf_r, 0.0, x_r, ALU.add, ALU.add)
                nc.sync.dma_start(
                    out_d[b, :, 1024 * tz + 512 * u : 1024 * tz + 512 * (u + 1)]
                    .rearrange("(h p) n -> p h n", h=2),
                    dst,
                )

        # ================= schedule =================
        load_x(0, 0, 1, split=True)
        load_x(0, 1, 2)
        load_x(0, 2, 4)
        load_x(1, 0, 2)
        load_x(1, 2, 4)

        # sample 0 projections
        for t in range(8):
            proj_tile(0, t)
            if t % 2 == 1:
                g_tile(0, t // 2)
        scores(0)

        # sample 1 projections with sample-0 attention interleaved (z and
        # final are pipelined one step apart so finals never wait on evac)
        for j in range(4):
            proj_tile(1, 2 * j)
            if j == 0:
                transpose_f(0)
            elif j >= 2:
                att_fin(0, j - 2)
            proj_tile(1, 2 * j + 1)
            g_tile(1, j)
            if j >= 1:
                att_z(0, j - 1)
        scores(1)
        att_fin(0, 2)
        att_z(0, 3)
        transpose_f(1)
        att_fin(0, 3)
        att_z(1, 0)
        att_z(1, 1)
        att_fin(1, 0)
        att_z(1, 2)
        att_fin(1, 1)
        att_z(1, 3)
        att_fin(1, 2)
        att_fin(1, 3)

    nc.compile()
    return nc


_CACHE = {}


def _prepare(inputs):
    """Fold BN into weights/biases and build per-core input maps."""

    def fold(w, bias, gamma, beta, mean, var):
        inv = gamma / np.sqrt(var + EPS)
        return (w * inv[:, None]).astype(np.float32), (
            beta + (bias - mean) * inv
        ).astype(np.float32)

    Wg, bg = fold(
        inputs["g_w"], inputs["g_b"], inputs["g_gamma"], inputs["g_beta"],
        inputs["g_mean"], inputs["g_var"],
    )
    Wth, bth = fold(
        inputs["th_w"], inputs["th_b"], inputs["th_gamma"], inputs["th_beta"],
        inputs["th_mean"], inputs["th_var"],
    )
    Wph, bph = fold(
        inputs["ph_w"], inputs["ph_b"], inputs["ph_gamma"], inputs["ph_beta"],
        inputs["ph_mean"], inputs["ph_var"],
    )
    Ww, bw = fold(
        inputs["w_w"], inputs["w_b"], inputs["w_gamma"], inputs["w_beta"],
        inputs["w_mean"], inputs["w_var"],
    )

    x = np.asarray(inputs["x"], dtype=np.float32).reshape(B, C, N)
    x_adj = (x + bw[None, :, None]).astype(ml_dtypes.bfloat16)
    bg_a = bg - Wg @ bw
    bth_a = bth - Wth @ bw
    bph_a = bph - Wph @ bw

    WgT = np.ascontiguousarray(Wg.T)  # (256, 128)
    wg_host = np.concatenate([WgT[0:128], WgT[128:256]], axis=1)  # (128, 256)
    WtpT = np.concatenate([Wth.T, Wph.T], axis=1)  # (256, 256)
    wtp_host = np.concatenate([WtpT[0:128], WtpT[128:256]], axis=1)  # (128, 512)
    btp_host = np.concatenate([bth_a, bph_a, bth_a, bph_a]).reshape(1, 512)
    ww_host = np.ascontiguousarray(Ww.T)  # (128, 256)

    wbf = np.zeros((128, 1280), dtype=ml_dtypes.bfloat16)
    wbf[:, 0:256] = wg_host
    wbf[:, 256:512] = ww_host
    wbf[:, 512:640] = np.eye(128, dtype=np.float32)
    wbf[0, 640:1152] = btp_host[0]
    wbf[0, 1152:1280] = 1.0

    wtp16 = np.zeros((128, 640), dtype=np.float16)
    wtp16[:, 0:512] = wtp_host
    wtp16[:, 512:640] = np.eye(128, dtype=np.float32)

    consts = {
        "wbf": wbf,
        "wtp16": wtp16,
        "bg": np.ascontiguousarray(bg_a.reshape(CI, 1), dtype=np.float32),
    }
    in_maps = []
    for i in range(NCORES):
        m = dict(consts)
        m["x"] = np.ascontiguousarray(x_adj[BPC * i : BPC * (i + 1)])
        in_maps.append(m)
    return in_maps


def _get_nc():
    if "nc" not in _CACHE:
        _CACHE["nc"] = _build_nc()
    return _CACHE["nc"]


def run(inputs, **kw):
    """Run on hardware; returns (full_output, BassKernelResults)."""
    nc = _get_nc()
    in_maps = _prepare(inputs)
    res = run_bass_kernel_spmd(nc, in_maps, list(range(NCORES)), **kw)
    out = np.concatenate(
        [
            np.asarray(res.results[i]["out"], dtype=np.float32).reshape(BPC, C, 64, 64)
            for i in range(NCORES)
        ],
        axis=0,
    )
    return np.ascontiguousarray(out), res


def kernel(**inputs):
    out, _ = run(inputs)
    return out
